# revision 79
# baseline (speedup 1.0000x reference)
"""ViT attention block (B=8, N=1024, dim=1024, heads=16, d_k=64) on 8 trn2 NeuronCores.

Sharding: data-parallel over batch (1 batch per core), weights replicated.
No collectives; each core computes its batch's full attention output.

Per-core algorithm (all matmuls contract over the partition dim):
  - host pre-transposes x[b] -> xT [dim, tokens]; host pre-tiles the Q,K
    part of w_qkv into contiguous [128,128] blocks (wqk[j,k]) for fast DMA.
  - QT/KT M-tile j: psum chunks [128,512] accumulate over k; head pair p
    lives in M-tiles j=p (Q) and j=8+p (K), two heads' d_k=64 rows stacked.
  - V = xT.T @ wv -> [tokens, 1024], stored with a constant-1 column per
    head (65 cols/head) so the PV matmul produces softmax row-sums free.
  - per pair/token-tile: S^T chunk [128,512] = KT-slice.T @ QT-slice
    (K=64; the two heads co-execute as PE row groups via tile_position).
    exp(scale*S^T) on ScalarE straight out of PSUM -> E^T bf16 (max-sub
    skipped: |scale*S| < ~2.5, exp-safe, softmax shift-invariant).
  - PV: attnT chunks [65,512] accumulate over m tiles; row 64 is the
    softmax denominator. h0 rows staged to stg[0:64] by VectorE, h1 rows
    DMA'd to stg[64:128] (partition shift), Z rows DMA'd to a [2,1024]
    tile, partition-broadcast back (SBUF->SBUF, no DRAM bounce), one
    reciprocal + one [128,1024] multiply -> attnT bf16.
  - final = attnT.T @ w_out + b_out per [128,512] psum chunk.

PSUM budget (8 banks, all tiles are single-bank [*,512] fp32):
  4x S^T rotation ("st") + 2x PV/p0-filler ("pv") + 2x V/QKT/proj ("acc").
The 4-deep S^T rotation lets exp(mt) drain while S^T(mt+1) issues, which
removes the exp <-> S^T PSUM serialization that dominated the old stalls.

Schedule: pair-0's slots use V-projection matmuls as filler; pairs 1..6
interleave QKT filler for pair p+1; PV for pair p-1 trails one pair.
Slot emission order: S^T, exp, PV, filler - keeps the critical chain at
the head of the in-order TensorE queue.
"""

import numpy as np
import ml_dtypes

import concourse.bass as bass
from concourse import bacc
import concourse.mybir as mybir
import concourse.tile as tile
from concourse.bass_utils import run_bass_kernel_spmd

P = 128
N_TOK = 1024
DIM = 1024
HEADS = 16
D_K = 64
N_CORES = 8
SCALE = D_K ** -0.5  # 0.125

NP_T = N_TOK // P    # 8 token tiles
DP = DIM // P        # 8 dim (contraction) tiles
NPAIRS = HEADS // 2  # 8 head pairs
VW = D_K + 1         # 65: V columns per head incl. ones column

BF16 = mybir.dt.bfloat16
F32 = mybir.dt.float32
FP8 = mybir.dt.float8e4
W8_SCALE = 32.0  # host scale on fp8 Q,K weights; folded into the exp scale


def build_program():
    nc = bacc.Bacc("TRN2", target_bir_lowering=False, debug=False)

    xT = nc.dram_tensor("xT", [DIM, N_TOK], BF16, kind="ExternalInput").ap()
    # fp8 operands for the QKV Q,K projections (DoubleRow: contraction of
    # 256 per pass at 2x rate).  Layouts put the two k-subtiles (s) in the
    # free dim: xT8[kk, q, (s n)] = xT[256kk+128s+q, n]; wqk8[j, q,
    # (kk s c)] = 32 * w_qkv[256kk+128s+q, 128j+c].  The x32 weight scale
    # keeps U(+-1/32) weights out of e4m3's subnormal range; it cancels
    # via the exp scale (softmax logits are just scaled by 1024).
    xT8 = nc.dram_tensor("xT8", [DP // 2, P, 2 * N_TOK], FP8,
                         kind="ExternalInput").ap()
    wqk8 = nc.dram_tensor("wqk8", [2 * DP, P, DP * P], FP8,
                          kind="ExternalInput").ap()
    wv = nc.dram_tensor("wv", [DIM, DIM], BF16, kind="ExternalInput").ap()
    wout = nc.dram_tensor("w_out", [DIM, DIM], BF16, kind="ExternalInput").ap()
    bout = nc.dram_tensor("b_out", [DIM], BF16, kind="ExternalInput").ap()
    out = nc.dram_tensor("out", [N_TOK, DIM], F32, kind="ExternalOutput").ap()
    # Z-row bounces (partition-broadcast DMA needs a DRAM source, and the
    # slow DVE reciprocal needs a wide-partition [128,16] layout).  The
    # whole Z chain is bf16 (DMAs don't convert dtypes): ~0.4% relative
    # noise on the softmax scale, well inside the error budget.
    z_dram = nc.dram_tensor("z_scratch", [2, N_TOK], BF16).ap()
    z2_dram = nc.dram_tensor("z2_scratch", [2, N_TOK], BF16).ap()

    with tile.TileContext(nc) as tc:
        with (
            tc.tile_pool(name="persist", bufs=1) as persist,
            tc.tile_pool(name="qkt", bufs=4) as qktp,
            tc.tile_pool(name="wqkp", bufs=4) as wqkp,
            tc.tile_pool(name="et", bufs=16) as etp,
            tc.tile_pool(name="stg", bufs=2) as stgp,
            tc.tile_pool(name="nrm", bufs=2) as nrmp,
            tc.tile_pool(name="ev", bufs=3) as evp,
            tc.tile_pool(name="pst", bufs=2, space="PSUM") as pstp,
            tc.tile_pool(name="ppv", bufs=2, space="PSUM") as ppvp,
            tc.tile_pool(name="pacc", bufs=2, space="PSUM") as paccp,
        ):
            # ---------------- persistent SBUF ----------------
            xT_sb = []
            wv_sb = []
            wout_sb = []
            v_sb = []      # per token-tile: [128, 16*65] bf16
            attnT_sb = []  # per pair: [128, 1024] bf16
            for j in range(NP_T):
                v_sb.append(persist.tile([P, HEADS * VW], BF16, tag=f"v{j}",
                                         name=f"v{j}"))
            for p in range(NPAIRS):
                attnT_sb.append(persist.tile([P, N_TOK], BF16, tag=f"attnT{p}",
                                             name=f"attnT{p}"))
            bias_bc = persist.tile([P, DIM], BF16, tag="bias", name="bias")

            # ---------------- input DMAs ----------------
            # Startup is DMA-bandwidth-bound (~6.5MB of inputs): order the
            # streams by first use.  The bias broadcast goes last (only
            # needed at the projection).
            w_tiles = {}   # j -> [128, 8*128] weight M-tile

            def fetch_w(j, queue=None):
                """Load M-tile j's fp8 Q/K weights ([128, 4*2*128]) in one
                DMA; DoubleRow pass kk is wt[:, kk*256:(kk+1)*256] viewed
                as [128, 2, 128]."""
                t = wqkp.tile([P, DP * P], FP8, tag="wqk", name=f"w{j}")
                (queue or nc.sync).dma_start(t[:], wqk8[j])
                w_tiles[j] = t
                return t

            # sync+scalar queues: fp8 xT8 (QKT) then bf16 xT (V);
            # gpsimd queue: pair-0 QK weight M-tiles, then wv, then wout.
            fetch_w(0, nc.gpsimd)
            xT8_sb = []
            for kk in range(DP // 2):
                t = persist.tile([P, 2 * N_TOK], FP8, tag=f"xT8_{kk}",
                                 name=f"xT8_{kk}")
                (nc.sync if kk % 2 == 0 else nc.scalar).dma_start(
                    t[:], xT8[kk])
                xT8_sb.append(t)
            fetch_w(DP, nc.gpsimd)
            for k in range(DP):
                t = persist.tile([P, N_TOK], BF16, tag=f"xT{k}", name=f"xT{k}")
                (nc.sync if k % 2 == 0 else nc.scalar).dma_start(
                    t[:], xT[k * P:(k + 1) * P, :])
                xT_sb.append(t)
            # wv tiles borrow the ET pool's 16-slot rotation: they occupy
            # slots 0-7 until pair-0's last V matmul; pair-1's ET tiles
            # (the 9th..16th "et" allocations after these) reuse the slots
            # exactly when wv dies.  Saves a dedicated 16KB/partition pool.
            for k in range(DP):
                t = etp.tile([P, DIM], BF16, tag="et", name=f"wv{k}")
                nc.gpsimd.dma_start(t[:], wv[k * P:(k + 1) * P, :])
                wv_sb.append(t)
            for k in range(DP):
                t = persist.tile([P, DIM], BF16, tag=f"wout{k}",
                                 name=f"wout{k}")
                nc.gpsimd.dma_start(t[:], wout[k * P:(k + 1) * P, :])
                wout_sb.append(t)
            bias_in = bass.AP(tensor=bout.tensor, offset=bout.offset,
                              ap=[[0, P]] + list(bout.ap))
            nc.gpsimd.dma_start(bias_bc[:], bias_in)
            # ones columns of V
            for j in range(NP_T):
                nc.vector.memset(
                    v_sb[j][:].rearrange("p (h x) -> p h x", x=VW)[:, :, D_K:],
                    1.0)

            # ---------------- helpers ----------------
            def qkt_mm(ps, j, kk, nh, start, stop):
                lhsT = w_tiles[j][:, kk * 256:(kk + 1) * 256].rearrange(
                    "p (s c) -> p s c", s=2)
                rhs = xT8_sb[kk][:].rearrange(
                    "p (s n) -> p s n", s=2)[:, :, nh * 512:(nh + 1) * 512]
                nc.tensor.matmul(ps[:], lhsT=lhsT, rhs=rhs,
                                 start=start, stop=stop,
                                 perf_mode=mybir.MatmulPerfMode.DoubleRow)

            def qkt_chunk(j, nh, dest):
                """Accumulate QKV^T M-tile j, column half nh into psum and
                cast into dest[:, nh*512:...]. 4 DoubleRow matmuls + cast."""
                ps = paccp.tile([P, 512], F32, tag="acc", name=f"psq{j}_{nh}")
                for kk in range(DP // 2):
                    qkt_mm(ps, j, kk, nh, kk == 0, kk == DP // 2 - 1)
                nc.vector.tensor_copy(out=dest[:, nh * 512:(nh + 1) * 512],
                                      in_=ps[:])

            def filler_step(pnext, mt, dest_qt, dest_kt, pool, ptag):
                """4 QKT matmuls toward pair pnext's QT/KT (32 mms over the
                8 slots of a pair). Slot mt covers chunk (j, nh) half
                `mt%2`; weight blocks are DMA'd on first use per (j,k).
                Pair 0 fillers use the idle PV pool; later pairs use the
                acc pool (V projection is done by then)."""
                j = pnext if mt < 4 else DP + pnext
                s = mt % 4
                nh, half = s // 2, s % 2
                key = ("fps", pnext, j, nh)
                if half == 0:
                    inflight[key] = pool.tile([P, 512], F32, tag=ptag,
                                              name=f"psf{j}_{nh}")
                ps = inflight[key]
                for kk in range(2 * half, 2 * half + 2):
                    qkt_mm(ps, j, kk, nh, kk == 0, kk == DP // 2 - 1)
                if half == 1:
                    dest = dest_qt if j < DP else dest_kt
                    nc.vector.tensor_copy(
                        out=dest[:, nh * 512:(nh + 1) * 512], in_=ps[:])
                    del inflight[key]
                    if nh == 1:
                        del w_tiles[j]

            def st_exp_step(p, mt, qt, kt):
                """4 S^T matmuls (2 heads x 2 col-halves; heads co-execute
                as PE row groups) + 2 exps into the ET tile.  The ET
                layout is nh-major ([nh][h][n]) so each exp reads one
                2-bank [128,1024] psum tile and writes contiguously; the
                ~200ns fixed ACT overhead is paid 2x instead of 4x."""
                et = etp.tile([P, 2 * N_TOK], BF16, tag="et", name=f"et{p}_{mt}")
                sts = {}
                for nh in range(2):
                    st = pstp.tile([P, 2 * 512], F32, tag="st",
                                   name=f"st{p}_{mt}_{nh}")
                    sts[nh] = st
                    for h in range(2):
                        nc.tensor.matmul(
                            st[:, h * 512:(h + 1) * 512],
                            lhsT=kt[h * D_K:(h + 1) * D_K,
                                    mt * P:(mt + 1) * P],
                            rhs=qt[h * D_K:(h + 1) * D_K,
                                   nh * 512:(nh + 1) * 512],
                            start=True, stop=True,
                            tile_position=(h * D_K, 0),
                        )
                for nh in range(2):
                    nc.scalar.activation(
                        et[:, nh * N_TOK:(nh + 1) * N_TOK],
                        sts[nh][:],
                        mybir.ActivationFunctionType.Exp,
                        scale=float(SCALE / (W8_SCALE * W8_SCALE)))
                et_tiles[(p, mt)] = et

            PV_ORDER = ((0, 0), (0, 1), (1, 0), (1, 1))
            PV_ORDER_H1F = ((1, 0), (1, 1), (0, 0), (0, 1))

            def pv_step(p, slot8, alloc=None, order=PV_ORDER):
                """4 PV matmuls for pair p: chunk (h, nh) accumulates all 8
                m-tiles across 2 slots. When a head's second chunk lands,
                its normalize pipeline (Z legs + multiply) is emitted so
                the per-pair normalize work is spread across slots.  The
                drain pass uses h1-first order so the h1 bounce-DMA path
                overlaps the drain and only h0's short path trails."""
                h, nh = order[slot8 // 2]
                half = slot8 % 2
                hg = 2 * p + h
                if half == 0:
                    name = f"pv{p}_{h}_{nh}"
                    inflight[("pv", p, h, nh)] = (
                        alloc([VW, 512], name) if alloc else
                        ppvp.tile([VW, 512], F32, tag="pv", name=name))
                pvt = inflight[("pv", p, h, nh)]
                for mt in range(4 * half, 4 * half + 4):
                    et = et_tiles[(p, mt)]
                    nc.tensor.matmul(
                        pvt[:],
                        lhsT=v_sb[mt][:, hg * VW:(hg + 1) * VW],
                        rhs=et[:, nh * N_TOK + h * 512:
                               nh * N_TOK + (h + 1) * 512],
                        start=(mt == 0), stop=(mt == NP_T - 1),
                    )
                if half == 1:
                    if ("hstg", p, h) not in inflight:
                        inflight[("hstg", p, h)] = stgp.tile(
                            [VW, N_TOK], BF16, tag=f"hstg{h}", bufs=2,
                            name=f"hstg{p}_{h}")
                    hstg = inflight[("hstg", p, h)]
                    cols = slice(nh * 512, (nh + 1) * 512)
                    nc.vector.tensor_copy(out=hstg[:, cols], in_=pvt[:])
                    del inflight[("pv", p, h, nh)]
                    if ("done", p) not in inflight:
                        inflight[("done", p)] = set()
                    done = inflight[("done", p)]
                    done.add((h, nh))
                    if {(h, 0), (h, 1)} <= done:
                        norm_head(p, h, hstg, drain=(order is PV_ORDER_H1F))
                    if len(done) == 4:
                        del inflight[("done", p)]
                        del inflight[("hstg", p, 0)]
                        del inflight[("hstg", p, 1)]
                        for mt in range(NP_T):
                            del et_tiles[(p, mt)]

            def norm_head(p, h, hstg, drain=False):
                """attnT[p] head h = staged PV rows * (1/Z_h).  Z row
                (partition 64 of the staging tile) -> DRAM -> [128,8] for
                a wide (fast) reciprocal -> DRAM -> partition-broadcast.
                h1's product bounces through a tile + DMA (VectorE can't
                shift partitions).  In the drain, the broadcast moves to
                the (idle) scalar queue so it isn't stuck behind the otmp
                DMA, which itself waits on the h1 multiply."""
                bcq = nc.scalar if drain else nc.gpsimd
                nc.sync.dma_start(z_dram[h:h + 1, :], hstg[D_K:VW, :])
                zsm = nrmp.tile([P, 8], BF16, tag="zsm", bufs=2,
                                name=f"zsm{p}_{h}")
                nc.gpsimd.dma_start(
                    zsm[:], z_dram[h].rearrange("(p i) -> p i", p=P))
                zsr = nrmp.tile([P, 8], BF16, tag="zsr", bufs=2,
                                name=f"zsr{p}_{h}")
                with nc.allow_low_precision(
                        reason="softmax denominators tolerate bf16"):
                    nc.vector.reciprocal(zsr[:], zsm[:])
                nc.sync.dma_start(
                    z2_dram[h].rearrange("(p i) -> p i", p=P), zsr[:])
                zrow = z2_dram[h:h + 1, :]
                rcp = nrmp.tile([D_K, N_TOK], BF16, tag="rcp", bufs=2,
                                name=f"rcp{p}_{h}")
                if drain:
                    # split the broadcast across two idle queues to halve
                    # its transfer latency on the critical tail
                    z_bc = bass.AP(tensor=zrow.tensor, offset=zrow.offset,
                                   ap=[[0, D_K // 2], list(zrow.ap)[-1]])
                    nc.scalar.dma_start(rcp[0:D_K // 2, :], z_bc)
                    nc.sync.dma_start(rcp[D_K // 2:D_K, :], z_bc)
                else:
                    z_bc = bass.AP(tensor=zrow.tensor, offset=zrow.offset,
                                   ap=[[0, D_K], list(zrow.ap)[-1]])
                    bcq.dma_start(rcp[:], z_bc)
                if h == 0:
                    nc.vector.tensor_mul(out=attnT_sb[p][0:D_K, :],
                                         in0=hstg[0:D_K, :], in1=rcp[:])
                else:
                    otmp = nrmp.tile([D_K, N_TOK], BF16, tag="otmp", bufs=2,
                                     name=f"otmp{p}")
                    nc.vector.tensor_mul(out=otmp[:], in0=hstg[0:D_K, :],
                                         in1=rcp[:])
                    nc.gpsimd.dma_start(attnT_sb[p][D_K:P, :], otmp[:])

            # ---------------- phase A: QKT(p0) ----------------
            inflight = {}
            et_tiles = {}
            qt_cur = qktp.tile([P, N_TOK], BF16, tag="qkt", name="qt0")
            kt_cur = qktp.tile([P, N_TOK], BF16, tag="qkt", name="kt0")
            for nh in range(2):
                qkt_chunk(0, nh, qt_cur)
            for nh in range(2):
                qkt_chunk(DP, nh, kt_cur)
            del w_tiles[0]
            del w_tiles[DP]

            # ---------------- phase B: pair loop ----------------
            for p in range(NPAIRS):
                qt_next = kt_next = None
                if p + 1 < NPAIRS:
                    qt_next = qktp.tile([P, N_TOK], BF16, tag="qkt",
                                        name=f"qt{p + 1}")
                    kt_next = qktp.tile([P, N_TOK], BF16, tag="qkt",
                                        name=f"kt{p + 1}")
                    fetch_w(p + 1)
                    fetch_w(DP + p + 1)
                for mt in range(NP_T):
                    st_exp_step(p, mt, qt_cur, kt_cur)
                    if p > 0:
                        pv_step(p - 1, mt)
                    if p == 0:
                        # V projection: 2 chunks (j=mt, nh=0/1) per slot
                        for nh in range(2):
                            ps = paccp.tile([P, 512], F32, tag="acc",
                                            name=f"psv{mt}_{nh}")
                            for k in range(DP):
                                nc.tensor.matmul(
                                    ps[:],
                                    lhsT=xT_sb[k][:, mt * P:(mt + 1) * P],
                                    rhs=wv_sb[k][:, nh * 512:(nh + 1) * 512],
                                    start=(k == 0), stop=(k == DP - 1),
                                )
                            vt = v_sb[mt][:].rearrange("p (h x) -> p h x",
                                                       x=VW)
                            nc.vector.tensor_copy(
                                out=vt[:, 8 * nh:8 * (nh + 1), :D_K],
                                in_=ps[:].rearrange("p (h d) -> p h d",
                                                    d=D_K),
                            )
                    if p + 1 < NPAIRS:
                        if p == 0:
                            filler_step(p + 1, mt, qt_next, kt_next,
                                        ppvp, "pv")
                        else:
                            filler_step(p + 1, mt, qt_next, kt_next,
                                        paccp, "acc")
                if p + 1 < NPAIRS:
                    qt_cur, kt_cur = qt_next, kt_next

            # ---------------- phase C: drain PV(7) + projection ----------------
            # All psum pools are free now: cycle chunks across them so
            # every bank-reuse wait lands on a long-finished chunk.  The
            # first 6 projection chunks accumulate p0..p6 while pair-7's
            # normalize chain completes; their p7 matmuls come last.
            cyc = [(pstp, "st"), (ppvp, "pv"), (paccp, "acc")]
            ci = [0]

            def phasec_tile(shape, name):
                pool, tg = cyc[ci[0] % 3]
                ci[0] += 1
                return pool.tile(shape, F32, tag=tg, name=name)

            for slot8 in range(NP_T):
                pv_step(NPAIRS - 1, slot8, alloc=phasec_tile,
                        order=PV_ORDER_H1F)

            def proj_mm(ps, j, nh, p):
                nc.tensor.matmul(
                    ps[:],
                    lhsT=attnT_sb[p][:, j * P:(j + 1) * P],
                    rhs=wout_sb[p][:, nh * 512:(nh + 1) * 512],
                    start=(p == 0), stop=(p == NPAIRS - 1),
                )

            def proj_evict(ps, j, nh):
                o = evp.tile([P, 512], F32, tag="out", name=f"o{j}_{nh}")
                nc.vector.tensor_add(out=o[:], in0=ps[:],
                                     in1=bias_bc[:, nh * 512:(nh + 1) * 512])
                nc.sync.dma_start(out[j * P:(j + 1) * P,
                                      nh * 512:(nh + 1) * 512], o[:])

            # Eight early chunks: the two 2-bank "st" slots each host two
            # projection chunks (independent psum banks, same pattern as
            # the S^T tiles), so all 8 psum banks accumulate p0..p6 while
            # pair-7's normalize chain completes; p7 matmuls come last.
            early = [(j, nh) for j in range(4) for nh in range(2)]
            stA = pstp.tile([P, 2 * 512], F32, tag="st", name="psoA")
            stB = pstp.tile([P, 2 * 512], F32, tag="st", name="psoB")
            assign = [stA[:, 0:512], stA[:, 512:1024],
                      stB[:, 0:512], stB[:, 512:1024],
                      ppvp.tile([P, 512], F32, tag="pv", name="psoP0"),
                      ppvp.tile([P, 512], F32, tag="pv", name="psoP1"),
                      paccp.tile([P, 512], F32, tag="acc", name="psoC0"),
                      paccp.tile([P, 512], F32, tag="acc", name="psoC1")]
            early_ps = {}
            for i, (j, nh) in enumerate(early):
                ps = assign[i]
                early_ps[(j, nh)] = ps
                for p in range(NPAIRS - 1):
                    proj_mm(ps, j, nh, p)
            for j, nh in early:
                ps = early_ps[(j, nh)]
                proj_mm(ps, j, nh, NPAIRS - 1)
                proj_evict(ps, j, nh)
            for j in range(4, NP_T):
                for nh in range(2):
                    ps = phasec_tile([P, 512], f"pso{j}_{nh}")
                    for p in range(NPAIRS):
                        proj_mm(ps, j, nh, p)
                    proj_evict(ps, j, nh)

    nc.compile()
    return nc


_NC_CACHE = None


def _get_program():
    global _NC_CACHE
    if _NC_CACHE is None:
        _NC_CACHE = build_program()
    return _NC_CACHE


def make_in_maps(x, w_qkv, w_out, b_out):
    bf = ml_dtypes.bfloat16
    f8 = ml_dtypes.float8_e4m3fn
    w_qkv = np.ascontiguousarray(w_qkv).astype(np.float32)
    # fp8 Q,K weights, scaled by 32 and tiled [j, q, (kk s c)] for
    # DoubleRow: element = 32 * w_qkv[256kk+128s+q, 128j+c]
    wqk8_t = np.ascontiguousarray(
        (w_qkv[:, :2 * DIM] * 32.0)
        .reshape(DP // 2, 2, P, 2 * DP, P).transpose(3, 2, 0, 1, 4)
        .reshape(2 * DP, P, DP * P)
    ).astype(f8)
    wv_c = np.ascontiguousarray(w_qkv[:, 2 * DIM:]).astype(bf)
    w_out_c = np.ascontiguousarray(w_out).astype(bf)
    b_out_c = np.ascontiguousarray(b_out).astype(bf)
    in_maps = []
    for b in range(N_CORES):
        xTb = np.ascontiguousarray(np.asarray(x[b]).T.astype(np.float32))
        # [kk, q, (s n)]: element = xT[256kk+128s+q, n]
        xT8b = np.ascontiguousarray(
            xTb.reshape(DP // 2, 2, P, N_TOK).transpose(0, 2, 1, 3)
            .reshape(DP // 2, P, 2 * N_TOK)).astype(f8)
        in_maps.append({
            "xT": xTb.astype(bf),
            "xT8": xT8b,
            "wqk8": wqk8_t,
            "wv": wv_c,
            "w_out": w_out_c,
            "b_out": b_out_c,
        })
    return in_maps


def kernel(x, w_qkv, w_out, b_out):
    nc = _get_program()
    in_maps = make_in_maps(x, w_qkv, w_out, b_out)
    res = run_bass_kernel_spmd(nc, in_maps, list(range(N_CORES)))
    outs = [np.asarray(r["out"], dtype=np.float32) for r in res.results]
    return np.stack(outs, axis=0)


# revision 80
# speedup vs baseline: 1.0354x; 1.0354x over previous
"""ViT attention block (B=8, N=1024, dim=1024, heads=16, d_k=64) on 8 trn2 NeuronCores.

Sharding: data-parallel over batch (1 batch per core), weights replicated.
No collectives; each core computes its batch's full attention output.

Per-core algorithm (all matmuls contract over the partition dim):
  - host pre-transposes x[b] -> xT [dim, tokens]; host pre-tiles the Q,K
    part of w_qkv into contiguous [128,128] blocks (wqk[j,k]) for fast DMA.
  - QT/KT M-tile j: psum chunks [128,512] accumulate over k; head pair p
    lives in M-tiles j=p (Q) and j=8+p (K), two heads' d_k=64 rows stacked.
  - V = xT.T @ wv -> [tokens, 1024], stored with a constant-1 column per
    head (65 cols/head) so the PV matmul produces softmax row-sums free.
  - per pair/token-tile: S^T chunk [128,512] = KT-slice.T @ QT-slice
    (K=64; the two heads co-execute as PE row groups via tile_position).
    exp(scale*S^T) on ScalarE straight out of PSUM -> E^T bf16 (max-sub
    skipped: |scale*S| < ~2.5, exp-safe, softmax shift-invariant).
  - PV: attnT chunks [65,512] accumulate over m tiles; row 64 is the
    softmax denominator. h0 rows staged to stg[0:64] by VectorE, h1 rows
    DMA'd to stg[64:128] (partition shift), Z rows DMA'd to a [2,1024]
    tile, partition-broadcast back (SBUF->SBUF, no DRAM bounce), one
    reciprocal + one [128,1024] multiply -> attnT bf16.
  - final = attnT.T @ w_out + b_out per [128,512] psum chunk.

PSUM budget (8 banks, all tiles are single-bank [*,512] fp32):
  4x S^T rotation ("st") + 2x PV/p0-filler ("pv") + 2x V/QKT/proj ("acc").
The 4-deep S^T rotation lets exp(mt) drain while S^T(mt+1) issues, which
removes the exp <-> S^T PSUM serialization that dominated the old stalls.

Schedule: pair-0's slots use V-projection matmuls as filler; pairs 1..6
interleave QKT filler for pair p+1; PV for pair p-1 trails one pair.
Slot emission order: S^T, exp, PV, filler - keeps the critical chain at
the head of the in-order TensorE queue.
"""

import numpy as np
import ml_dtypes

import concourse.bass as bass
from concourse import bacc
import concourse.mybir as mybir
import concourse.tile as tile
from concourse.bass_utils import run_bass_kernel_spmd

P = 128
N_TOK = 1024
DIM = 1024
HEADS = 16
D_K = 64
N_CORES = 8
SCALE = D_K ** -0.5  # 0.125

NP_T = N_TOK // P    # 8 token tiles
DP = DIM // P        # 8 dim (contraction) tiles
NPAIRS = HEADS // 2  # 8 head pairs
VW = D_K + 1         # 65: V columns per head incl. ones column

BF16 = mybir.dt.bfloat16
F32 = mybir.dt.float32
FP8 = mybir.dt.float8e4
W8_SCALE = 32.0  # host scale on fp8 Q,K weights; folded into the exp scale


def build_program():
    nc = bacc.Bacc("TRN2", target_bir_lowering=False, debug=False)

    xT = nc.dram_tensor("xT", [DIM, N_TOK], BF16, kind="ExternalInput").ap()
    # fp8 operands for the QKV Q,K projections (DoubleRow: contraction of
    # 256 per pass at 2x rate).  Layouts put the two k-subtiles (s) in the
    # free dim: xT8[kk, q, (s n)] = xT[256kk+128s+q, n]; wqk8[j, q,
    # (kk s c)] = 32 * w_qkv[256kk+128s+q, 128j+c].  The x32 weight scale
    # keeps U(+-1/32) weights out of e4m3's subnormal range; it cancels
    # via the exp scale (softmax logits are just scaled by 1024).
    xT8 = nc.dram_tensor("xT8", [DP // 2, P, 2 * N_TOK], FP8,
                         kind="ExternalInput").ap()
    wqk8 = nc.dram_tensor("wqk8", [2 * DP, P, DP * P], FP8,
                          kind="ExternalInput").ap()
    wv = nc.dram_tensor("wv", [DIM, DIM], BF16, kind="ExternalInput").ap()
    wout = nc.dram_tensor("w_out", [DIM, DIM], BF16, kind="ExternalInput").ap()
    bout = nc.dram_tensor("b_out", [DIM], BF16, kind="ExternalInput").ap()
    out = nc.dram_tensor("out", [N_TOK, DIM], F32, kind="ExternalOutput").ap()
    # Z-row bounces (partition-broadcast DMA needs a DRAM source, and the
    # slow DVE reciprocal needs a wide-partition [128,16] layout).  The
    # whole Z chain is bf16 (DMAs don't convert dtypes): ~0.4% relative
    # noise on the softmax scale, well inside the error budget.
    z_dram = nc.dram_tensor("z_scratch", [2, N_TOK], BF16).ap()
    z2_dram = nc.dram_tensor("z2_scratch", [2, N_TOK], BF16).ap()

    with tile.TileContext(nc) as tc:
        with (
            tc.tile_pool(name="persist", bufs=1) as persist,
            tc.tile_pool(name="qkt", bufs=4) as qktp,
            tc.tile_pool(name="wqkp", bufs=4) as wqkp,
            tc.tile_pool(name="et", bufs=16) as etp,
            tc.tile_pool(name="stg", bufs=2) as stgp,
            tc.tile_pool(name="nrm", bufs=2) as nrmp,
            tc.tile_pool(name="ev", bufs=3) as evp,
            tc.tile_pool(name="pst", bufs=2, space="PSUM") as pstp,
            tc.tile_pool(name="ppv", bufs=2, space="PSUM") as ppvp,
            tc.tile_pool(name="pacc", bufs=2, space="PSUM") as paccp,
        ):
            # ---------------- persistent SBUF ----------------
            xT_sb = []
            wv_sb = []
            wout_sb = []
            v_sb = []      # per token-tile: [128, 16*65] bf16
            attnT_sb = []  # per pair: [128, 1024] bf16
            for j in range(NP_T):
                v_sb.append(persist.tile([P, HEADS * VW], BF16, tag=f"v{j}",
                                         name=f"v{j}"))
            for p in range(NPAIRS):
                attnT_sb.append(persist.tile([P, N_TOK], BF16, tag=f"attnT{p}",
                                             name=f"attnT{p}"))
            bias_bc = persist.tile([P, DIM], BF16, tag="bias", name="bias")

            # ---------------- input DMAs ----------------
            # Startup is DMA-bandwidth-bound (~6.5MB of inputs): order the
            # streams by first use.  The bias broadcast goes last (only
            # needed at the projection).
            w_tiles = {}   # j -> [128, 8*128] weight M-tile

            def fetch_w(j, queue=None):
                """Load M-tile j's fp8 Q/K weights ([128, 4*2*128]) in one
                DMA; DoubleRow pass kk is wt[:, kk*256:(kk+1)*256] viewed
                as [128, 2, 128]."""
                t = wqkp.tile([P, DP * P], FP8, tag="wqk", name=f"w{j}")
                (queue or nc.sync).dma_start(t[:], wqk8[j])
                w_tiles[j] = t
                return t

            # sync+scalar queues: fp8 xT8 (QKT) then bf16 xT (V);
            # gpsimd queue: pair-0 QK weight M-tiles, then wv, then wout.
            fetch_w(0, nc.gpsimd)
            xT8_sb = []
            for kk in range(DP // 2):
                t = persist.tile([P, 2 * N_TOK], FP8, tag=f"xT8_{kk}",
                                 name=f"xT8_{kk}")
                (nc.sync if kk % 2 == 0 else nc.scalar).dma_start(
                    t[:], xT8[kk])
                xT8_sb.append(t)
            fetch_w(DP, nc.gpsimd)
            for k in range(DP):
                t = persist.tile([P, N_TOK], BF16, tag=f"xT{k}", name=f"xT{k}")
                (nc.sync if k % 2 == 0 else nc.scalar).dma_start(
                    t[:], xT[k * P:(k + 1) * P, :])
                xT_sb.append(t)
            # wv tiles borrow the ET pool's 16-slot rotation: they occupy
            # slots 0-7 until pair-0's last V matmul; pair-1's ET tiles
            # (the 9th..16th "et" allocations after these) reuse the slots
            # exactly when wv dies.  Saves a dedicated 16KB/partition pool.
            for k in range(DP):
                t = etp.tile([P, DIM], BF16, tag="et", name=f"wv{k}")
                nc.gpsimd.dma_start(t[:], wv[k * P:(k + 1) * P, :])
                wv_sb.append(t)
            for k in range(DP):
                t = persist.tile([P, DIM], BF16, tag=f"wout{k}",
                                 name=f"wout{k}")
                nc.gpsimd.dma_start(t[:], wout[k * P:(k + 1) * P, :])
                wout_sb.append(t)
            bias_in = bass.AP(tensor=bout.tensor, offset=bout.offset,
                              ap=[[0, P]] + list(bout.ap))
            nc.gpsimd.dma_start(bias_bc[:], bias_in)
            # ones columns of V
            for j in range(NP_T):
                nc.vector.memset(
                    v_sb[j][:].rearrange("p (h x) -> p h x", x=VW)[:, :, D_K:],
                    1.0)

            # ---------------- helpers ----------------
            def qkt_mm(ps, j, kk, nh, start, stop):
                lhsT = w_tiles[j][:, kk * 256:(kk + 1) * 256].rearrange(
                    "p (s c) -> p s c", s=2)
                rhs = xT8_sb[kk][:].rearrange(
                    "p (s n) -> p s n", s=2)[:, :, nh * 512:(nh + 1) * 512]
                nc.tensor.matmul(ps[:], lhsT=lhsT, rhs=rhs,
                                 start=start, stop=stop,
                                 perf_mode=mybir.MatmulPerfMode.DoubleRow)

            def qkt_chunk(j, nh, dest):
                """Accumulate QKV^T M-tile j, column half nh into psum and
                cast into dest[:, nh*512:...]. 4 DoubleRow matmuls + cast."""
                ps = paccp.tile([P, 512], F32, tag="acc", name=f"psq{j}_{nh}")
                for kk in range(DP // 2):
                    qkt_mm(ps, j, kk, nh, kk == 0, kk == DP // 2 - 1)
                nc.vector.tensor_copy(out=dest[:, nh * 512:(nh + 1) * 512],
                                      in_=ps[:])

            def filler_step(pnext, mt, dest_qt, dest_kt, pool, ptag):
                """4 QKT matmuls toward pair pnext's QT/KT (32 mms over the
                8 slots of a pair). Slot mt covers chunk (j, nh) half
                `mt%2`; weight blocks are DMA'd on first use per (j,k).
                Pair 0 fillers use the idle PV pool; later pairs use the
                acc pool (V projection is done by then)."""
                j = pnext if mt < 4 else DP + pnext
                s = mt % 4
                nh, half = s // 2, s % 2
                key = ("fps", pnext, j, nh)
                if half == 0:
                    inflight[key] = pool.tile([P, 512], F32, tag=ptag,
                                              name=f"psf{j}_{nh}")
                ps = inflight[key]
                for kk in range(2 * half, 2 * half + 2):
                    qkt_mm(ps, j, kk, nh, kk == 0, kk == DP // 2 - 1)
                if half == 1:
                    dest = dest_qt if j < DP else dest_kt
                    nc.vector.tensor_copy(
                        out=dest[:, nh * 512:(nh + 1) * 512], in_=ps[:])
                    del inflight[key]
                    if nh == 1:
                        del w_tiles[j]

            def st_exp_step(p, mt, qt, kt):
                """4 S^T matmuls (2 heads x 2 col-halves; heads co-execute
                as PE row groups) + 2 exps into the ET tile.  The ET
                layout is nh-major ([nh][h][n]) so each exp reads one
                2-bank [128,1024] psum tile and writes contiguously; the
                ~200ns fixed ACT overhead is paid 2x instead of 4x."""
                et = etp.tile([P, 2 * N_TOK], BF16, tag="et", name=f"et{p}_{mt}")
                sts = {}
                for nh in range(2):
                    st = pstp.tile([P, 2 * 512], F32, tag="st",
                                   name=f"st{p}_{mt}_{nh}")
                    sts[nh] = st
                    for h in range(2):
                        nc.tensor.matmul(
                            st[:, h * 512:(h + 1) * 512],
                            lhsT=kt[h * D_K:(h + 1) * D_K,
                                    mt * P:(mt + 1) * P],
                            rhs=qt[h * D_K:(h + 1) * D_K,
                                   nh * 512:(nh + 1) * 512],
                            start=True, stop=True,
                            tile_position=(h * D_K, 0),
                        )
                for nh in range(2):
                    nc.scalar.activation(
                        et[:, nh * N_TOK:(nh + 1) * N_TOK],
                        sts[nh][:],
                        mybir.ActivationFunctionType.Exp,
                        scale=float(SCALE / (W8_SCALE * W8_SCALE)))
                et_tiles[(p, mt)] = et

            PV_ORDER = ((0, 0), (0, 1), (1, 0), (1, 1))
            PV_ORDER_H1F = ((1, 0), (1, 1), (0, 0), (0, 1))

            def pv_step(p, slot8, alloc=None, order=PV_ORDER):
                """4 PV matmuls for pair p: chunk (h, nh) accumulates all 8
                m-tiles across 2 slots. When a head's second chunk lands,
                its normalize pipeline (Z legs + multiply) is emitted so
                the per-pair normalize work is spread across slots.  The
                drain pass uses h1-first order so the h1 bounce-DMA path
                overlaps the drain and only h0's short path trails."""
                h, nh = order[slot8 // 2]
                half = slot8 % 2
                hg = 2 * p + h
                if half == 0:
                    name = f"pv{p}_{h}_{nh}"
                    inflight[("pv", p, h, nh)] = (
                        alloc([VW, 512], name) if alloc else
                        ppvp.tile([VW, 512], F32, tag="pv", name=name))
                pvt = inflight[("pv", p, h, nh)]
                for mt in range(4 * half, 4 * half + 4):
                    et = et_tiles[(p, mt)]
                    nc.tensor.matmul(
                        pvt[:],
                        lhsT=v_sb[mt][:, hg * VW:(hg + 1) * VW],
                        rhs=et[:, nh * N_TOK + h * 512:
                               nh * N_TOK + (h + 1) * 512],
                        start=(mt == 0), stop=(mt == NP_T - 1),
                    )
                if half == 1:
                    if ("hstg", p, h) not in inflight:
                        inflight[("hstg", p, h)] = stgp.tile(
                            [VW, N_TOK], BF16, tag=f"hstg{h}", bufs=2,
                            name=f"hstg{p}_{h}")
                    hstg = inflight[("hstg", p, h)]
                    cols = slice(nh * 512, (nh + 1) * 512)
                    nc.vector.tensor_copy(out=hstg[:, cols], in_=pvt[:])
                    del inflight[("pv", p, h, nh)]
                    if ("done", p) not in inflight:
                        inflight[("done", p)] = set()
                    done = inflight[("done", p)]
                    done.add((h, nh))
                    if {(h, 0), (h, 1)} <= done:
                        norm_head(p, h, hstg, drain=(order is PV_ORDER_H1F))
                    if len(done) == 4:
                        del inflight[("done", p)]
                        del inflight[("hstg", p, 0)]
                        del inflight[("hstg", p, 1)]
                        for mt in range(NP_T):
                            del et_tiles[(p, mt)]

            def norm_head(p, h, hstg, drain=False):
                """attnT[p] head h = staged PV rows * (1/Z_h).  Z row
                (partition 64 of the staging tile) -> DRAM -> [128,8] for
                a wide (fast) reciprocal -> DRAM -> partition-broadcast.
                h1's product bounces through a tile + DMA (VectorE can't
                shift partitions).  In the drain, the broadcast moves to
                the (idle) scalar queue so it isn't stuck behind the otmp
                DMA, which itself waits on the h1 multiply."""
                bcq = nc.scalar if drain else nc.gpsimd
                nc.sync.dma_start(z_dram[h:h + 1, :], hstg[D_K:VW, :])
                zsm = nrmp.tile([P, 8], BF16, tag="zsm", bufs=2,
                                name=f"zsm{p}_{h}")
                nc.gpsimd.dma_start(
                    zsm[:], z_dram[h].rearrange("(p i) -> p i", p=P))
                zsr = nrmp.tile([P, 8], BF16, tag="zsr", bufs=2,
                                name=f"zsr{p}_{h}")
                with nc.allow_low_precision(
                        reason="softmax denominators tolerate bf16"):
                    nc.vector.reciprocal(zsr[:], zsm[:])
                nc.sync.dma_start(
                    z2_dram[h].rearrange("(p i) -> p i", p=P), zsr[:])
                zrow = z2_dram[h:h + 1, :]
                rcp = nrmp.tile([D_K, N_TOK], BF16, tag="rcp", bufs=2,
                                name=f"rcp{p}_{h}")
                if drain:
                    # split the broadcast across two idle queues to halve
                    # its transfer latency on the critical tail
                    z_bc = bass.AP(tensor=zrow.tensor, offset=zrow.offset,
                                   ap=[[0, D_K // 2], list(zrow.ap)[-1]])
                    nc.scalar.dma_start(rcp[0:D_K // 2, :], z_bc)
                    nc.sync.dma_start(rcp[D_K // 2:D_K, :], z_bc)
                else:
                    z_bc = bass.AP(tensor=zrow.tensor, offset=zrow.offset,
                                   ap=[[0, D_K], list(zrow.ap)[-1]])
                    bcq.dma_start(rcp[:], z_bc)
                if h == 0:
                    nc.vector.tensor_mul(out=attnT_sb[p][0:D_K, :],
                                         in0=hstg[0:D_K, :], in1=rcp[:])
                else:
                    otmp = nrmp.tile([D_K, N_TOK], BF16, tag="otmp", bufs=2,
                                     name=f"otmp{p}")
                    nc.vector.tensor_mul(out=otmp[:], in0=hstg[0:D_K, :],
                                         in1=rcp[:])
                    nc.gpsimd.dma_start(attnT_sb[p][D_K:P, :], otmp[:])

            # ---------------- phase A: QKT(p0) ----------------
            inflight = {}
            et_tiles = {}
            qt_cur = qktp.tile([P, N_TOK], BF16, tag="qkt", name="qt0")
            kt_cur = qktp.tile([P, N_TOK], BF16, tag="qkt", name="kt0")
            for nh in range(2):
                qkt_chunk(0, nh, qt_cur)
            for nh in range(2):
                qkt_chunk(DP, nh, kt_cur)
            del w_tiles[0]
            del w_tiles[DP]

            # ---------------- phase B: pair loop ----------------
            for p in range(NPAIRS):
                qt_next = kt_next = None
                if p + 1 < NPAIRS:
                    qt_next = qktp.tile([P, N_TOK], BF16, tag="qkt",
                                        name=f"qt{p + 1}")
                    kt_next = qktp.tile([P, N_TOK], BF16, tag="qkt",
                                        name=f"kt{p + 1}")
                    fetch_w(p + 1)
                    fetch_w(DP + p + 1)
                for mt in range(NP_T):
                    st_exp_step(p, mt, qt_cur, kt_cur)
                    if p > 0:
                        pv_step(p - 1, mt)
                    if p == 0:
                        # V projection: 2 chunks (j=mt, nh=0/1) per slot
                        for nh in range(2):
                            ps = paccp.tile([P, 512], F32, tag="acc",
                                            name=f"psv{mt}_{nh}")
                            for k in range(DP):
                                nc.tensor.matmul(
                                    ps[:],
                                    lhsT=xT_sb[k][:, mt * P:(mt + 1) * P],
                                    rhs=wv_sb[k][:, nh * 512:(nh + 1) * 512],
                                    start=(k == 0), stop=(k == DP - 1),
                                )
                            vt = v_sb[mt][:].rearrange("p (h x) -> p h x",
                                                       x=VW)
                            nc.vector.tensor_copy(
                                out=vt[:, 8 * nh:8 * (nh + 1), :D_K],
                                in_=ps[:].rearrange("p (h d) -> p h d",
                                                    d=D_K),
                            )
                    if p + 1 < NPAIRS:
                        if p == 0:
                            filler_step(p + 1, mt, qt_next, kt_next,
                                        ppvp, "pv")
                        else:
                            filler_step(p + 1, mt, qt_next, kt_next,
                                        paccp, "acc")
                if p + 1 < NPAIRS:
                    qt_cur, kt_cur = qt_next, kt_next

            # ---------------- phase C: drain PV(7) + projection ----------------
            # All psum pools are free now: cycle chunks across them so
            # every bank-reuse wait lands on a long-finished chunk.  The
            # first 6 projection chunks accumulate p0..p6 while pair-7's
            # normalize chain completes; their p7 matmuls come last.
            cyc = [(pstp, "st"), (ppvp, "pv"), (paccp, "acc")]
            ci = [0]

            def phasec_tile(shape, name):
                pool, tg = cyc[ci[0] % 3]
                ci[0] += 1
                return pool.tile(shape, F32, tag=tg, name=name)

            for slot8 in range(NP_T):
                pv_step(NPAIRS - 1, slot8, alloc=phasec_tile,
                        order=PV_ORDER_H1F)

            def proj_mm(ps, j, nh, p):
                nc.tensor.matmul(
                    ps[:],
                    lhsT=attnT_sb[p][:, j * P:(j + 1) * P],
                    rhs=wout_sb[p][:, nh * 512:(nh + 1) * 512],
                    start=(p == 0), stop=(p == NPAIRS - 1),
                )

            def proj_evict(ps, j, nh):
                o = evp.tile([P, 512], F32, tag="out", name=f"o{j}_{nh}")
                nc.vector.tensor_add(out=o[:], in0=ps[:],
                                     in1=bias_bc[:, nh * 512:(nh + 1) * 512])
                nc.sync.dma_start(out[j * P:(j + 1) * P,
                                      nh * 512:(nh + 1) * 512], o[:])

            early = [(0, 0), (0, 1), (1, 0), (1, 1), (2, 0), (2, 1)]
            early_ps = {}
            for j, nh in early:
                ps = phasec_tile([P, 512], f"pso{j}_{nh}")
                early_ps[(j, nh)] = ps
                for p in range(NPAIRS - 1):
                    proj_mm(ps, j, nh, p)
            for j, nh in early:
                ps = early_ps[(j, nh)]
                proj_mm(ps, j, nh, NPAIRS - 1)
                proj_evict(ps, j, nh)
            for j in range(3, NP_T):
                for nh in range(2):
                    ps = phasec_tile([P, 512], f"pso{j}_{nh}")
                    for p in range(NPAIRS):
                        proj_mm(ps, j, nh, p)
                    proj_evict(ps, j, nh)

    nc.compile()
    return nc


_NC_CACHE = None


def _get_program():
    global _NC_CACHE
    if _NC_CACHE is None:
        _NC_CACHE = build_program()
    return _NC_CACHE


def make_in_maps(x, w_qkv, w_out, b_out):
    bf = ml_dtypes.bfloat16
    f8 = ml_dtypes.float8_e4m3fn
    w_qkv = np.ascontiguousarray(w_qkv).astype(np.float32)
    # fp8 Q,K weights, scaled by 32 and tiled [j, q, (kk s c)] for
    # DoubleRow: element = 32 * w_qkv[256kk+128s+q, 128j+c]
    wqk8_t = np.ascontiguousarray(
        (w_qkv[:, :2 * DIM] * 32.0)
        .reshape(DP // 2, 2, P, 2 * DP, P).transpose(3, 2, 0, 1, 4)
        .reshape(2 * DP, P, DP * P)
    ).astype(f8)
    wv_c = np.ascontiguousarray(w_qkv[:, 2 * DIM:]).astype(bf)
    w_out_c = np.ascontiguousarray(w_out).astype(bf)
    b_out_c = np.ascontiguousarray(b_out).astype(bf)
    in_maps = []
    for b in range(N_CORES):
        xTb = np.ascontiguousarray(np.asarray(x[b]).T.astype(np.float32))
        # [kk, q, (s n)]: element = xT[256kk+128s+q, n]
        xT8b = np.ascontiguousarray(
            xTb.reshape(DP // 2, 2, P, N_TOK).transpose(0, 2, 1, 3)
            .reshape(DP // 2, P, 2 * N_TOK)).astype(f8)
        in_maps.append({
            "xT": xTb.astype(bf),
            "xT8": xT8b,
            "wqk8": wqk8_t,
            "wv": wv_c,
            "w_out": w_out_c,
            "b_out": b_out_c,
        })
    return in_maps


def kernel(x, w_qkv, w_out, b_out):
    nc = _get_program()
    in_maps = make_in_maps(x, w_qkv, w_out, b_out)
    res = run_bass_kernel_spmd(nc, in_maps, list(range(N_CORES)))
    outs = [np.asarray(r["out"], dtype=np.float32) for r in res.results]
    return np.stack(outs, axis=0)
